# revision 30
# baseline (speedup 1.0000x reference)
"""BNAF forward (B=2048, D=8, H=512, 4 masked layers) on 8 TRN2 NeuronCores.

Strategy
--------
Pure data parallel: batch is split 256/core; the small weights are replicated.

Math: the BNAF log-det recursion collapses in exp space: exp(logdet diag
blocks) == diag blocks of the normalized weight, exp(tanh logdet) == 1-h^2,
so the flow is a chain of positive block-diag matmuls with one log at the
end (2-op DVE fast-log).  The norm scale s=exp(logg)/||v|| is folded
input-side into the next layer's G-flow weights (wd form).  The sech^2
factor is applied as (h^2-1) -- the sign flips cancel across the even
number of layers (with the matching (h4^2-1) fold at L4).

Tile layout notes (trn2 Tile framework tracks deps at TILE granularity, so
false-sharing serializes):
- per-(layer,chunk) PSUM tiles for the h-path matmuls so chunk c+1's MMs
  don't wait on chunk c's tanh (whole-tile WAR).
- weights live in vtO (full rows, DMA-only writers) + vtD (the 4 diag
  128-blocks, strided DMA) per layer, so norm/exp work on vtD never blocks
  reads of the raw off-diag blocks.
- norm^2 columnize is 10 direct lhsT=v^2-window matmuls (no row-sum /
  transpose machinery); one-step Newton rsqrt from a magic seed.
"""

import numpy as np

TRACE = False          # set by test.py for profiling runs
LAST_RESULTS = None    # BassKernelResults stash for test.py

_CACHE = {}

P = 128
BC = 256          # batch per core
H = 512
NCORE = 8
MAGIC = 0x5f3759df

# smalls layout: first the exp-batch block (one ACT op), then the rest
_SM = {}
_off = 0
for _name, _w in [("w1dg", 4), ("w4dg", 4), ("lg1", 4), ("lg2", 4),
                  ("lg3", 4), ("lg4c", 1), ("w1n", 32), ("w4t", 32),  # exp blk
                  ("b4c", 1), ("b1", 4), ("b2", 4), ("b3", 4),
                  ("ident", 128),
                  ("md1n", 32), ("mo1n", 32), ("md4t", 32), ("mo4t", 32)]:
    _SM[_name] = (_off, _off + _w)
    _off += _w
SMALL_W = _off
EXPW = _SM["w4t"][1]           # width of the exp block (85)

FASTLN_A = float(np.log(2.0) / (1 << 23))
FASTLN_B = float((0.0430 - 127.0) * np.log(2.0))


def _build():
    import concourse.bacc as bacc
    import concourse.mybir as mybir
    import concourse.tile as tile
    from concourse.bass import AP
    from contextlib import ExitStack

    f32 = mybir.dt.float32
    u32 = mybir.dt.uint32
    i32 = mybir.dt.int32
    bf16 = mybir.dt.bfloat16
    fp16 = mybir.dt.float16
    E = mybir.ActivationFunctionType
    ALU = mybir.AluOpType

    nc = bacc.Bacc("TRN2", target_bir_lowering=False, debug=False,
                   enable_asserts=False, num_devices=NCORE)

    t = {}
    t["xT"] = nc.dram_tensor("xT", (8, BC), fp16, kind="ExternalInput").ap()
    t["w2T"] = nc.dram_tensor("w2T", (H, H), fp16, kind="ExternalInput").ap()
    t["w3T"] = nc.dram_tensor("w3T", (H, H), fp16, kind="ExternalInput").ap()
    t["smalls"] = nc.dram_tensor("smalls", (P, SMALL_W), f32, kind="ExternalInput").ap()
    t["hT_out"] = nc.dram_tensor("hT_out", (8, BC), f32, kind="ExternalOutput").ap()
    t["sldT_out"] = nc.dram_tensor("sldT_out", (8, BC), f32, kind="ExternalOutput").ap()

    def mm(out, lhsT, rhs, **kw):
        nc.tensor.matmul(out, lhsT, rhs, **kw)

    def winap(base_tile, p0, np_, col0, n, stride, w):
        """[np_ parts at p0] x (n windows of w cols, stride apart, from col0)."""
        base = base_tile[p0:p0 + np_, col0:col0 + w]
        return AP(base.tensor, base.offset,
                  [[base.ap[0][0], np_], [stride, n], [1, w]])

    with tile.TileContext(nc) as tc, ExitStack() as ctx:
        wgt = ctx.enter_context(tc.tile_pool(name="wgt", bufs=1))
        scr = ctx.enter_context(tc.tile_pool(name="scr", bufs=4))
        psN = ctx.enter_context(tc.tile_pool(name="psN", bufs=2, space="PSUM"))
        pzc = ctx.enter_context(tc.tile_pool(name="pzc", bufs=4, space="PSUM"))
        pzg = ctx.enter_context(tc.tile_pool(name="pzg", bufs=1, space="PSUM"))

        act = nc.scalar.activation
        cp = nc.vector.tensor_copy
        ts = nc.vector.tensor_scalar
        stt = nc.vector.scalar_tensor_tensor
        mul = nc.vector.tensor_mul
        tt = nc.vector.tensor_tensor
        gtt = nc.gpsimd.tensor_tensor
        gms = nc.gpsimd.memset

        # ---- dummy ACT at t0 pulls the single exp_and_others table load ---
        dmy = wgt.tile([P, 1], f32, name="dmy", tag="dmy")
        dmyo = wgt.tile([P, 1], f32, name="dmyo", tag="dmyo")
        nc.vector.memset(dmy, 0.0)
        act(dmyo, dmy, E.Exp)

        # ---- input DMAs ---------------------------------------------------
        # vtO: full rows (raw W, DMA is the only writer).  vtD: the four
        # diag 128-blocks per layer, chunk c at cols [128c, 128c+128).
        smalls = wgt.tile([P, SMALL_W], f32, name="smalls_t", tag="smalls_t")
        xT = wgt.tile([8, BC], fp16, name="xT_t", tag="xT_t")
        vtO = {l: wgt.tile([P, 4 * H], fp16, name=f"vtO{l}", tag=f"vtO{l}")
               for l in (2, 3)}
        # one diag tile for both layers: block (l, c) at col 512(l-2)+128c
        vtDt = wgt.tile([P, 8 * P], fp16, name="vtD", tag="vtD")
        vtD = {2: vtDt[:, 0:4 * P], 3: vtDt[:, 4 * P:8 * P]}
        nc.sync.dma_start(smalls, t["smalls"])
        nc.gpsimd.dma_start(vtD[2], AP(t["w2T"].tensor, 0,
                                       [[H, P], [P * H + P, 4], [1, P]]))
        nc.gpsimd.dma_start(vtD[3], AP(t["w3T"].tensor, 0,
                                       [[H, P], [P * H + P, 4], [1, P]]))
        for hh in (0, 1):
            src = AP(t["w2T"].tensor, hh * 2 * P * H,
                     [[H, P], [P * H, 2], [1, H]])
            nc.sync.dma_start(vtO[2][:, 2 * H * hh:2 * H * hh + 2 * H], src)
        nc.sync.dma_start(xT, t["xT"])
        for hh in (0, 1):
            src = AP(t["w3T"].tensor, hh * 2 * P * H,
                     [[H, P], [P * H, 2], [1, H]])
            nc.gpsimd.dma_start(vtO[3][:, 2 * H * hh:2 * H * hh + 2 * H], src)

        def sm(name):
            a, b = _SM[name]
            return smalls[:, a:b]

        ident = sm("ident")
        b4 = smalls[0:8, _SM["b4c"][0]:_SM["b4c"][1]]

        # ---- constants ----------------------------------------------------
        ones2f = wgt.tile([P, 2], f32, name="ones2f", tag="ones2f")
        ones2 = wgt.tile([P, 2], fp16, name="ones2", tag="ones2")
        magict = wgt.tile([P, 5], u32, name="magict", tag="magict")
        gms(ones2f, 1.0)
        gms(magict, MAGIC)
        cp(ones2, ones2f)

        # ---- diag prep: zero LL quadrants, exp the 64-blocks (both layers)
        gms(winap(vtDt, 64, 64, 0, 8, P, 64), 0.0)
        dA = winap(vtDt, 0, 64, 0, 8, P, 64)
        dB = winap(vtDt, 64, 64, 64, 8, P, 64)
        act(dA, dA, E.Exp)
        act(dB, dB, E.Exp)

        # ---- one exp over all the small weight pieces ---------------------
        esm = wgt.tile([P, EXPW], f32, name="esm", tag="esm")
        act(esm, smalls[:, 0:EXPW], E.Exp)
        e1n = esm[:, _SM["w1n"][0]:_SM["w1n"][1]]
        e4t = esm[:, _SM["w4t"][0]:_SM["w4t"][1]]
        e1d = esm[:, _SM["w1dg"][0]:_SM["w1dg"][1]]
        eg = {1: esm[:, _SM["lg1"][0]:_SM["lg1"][1]],
              2: esm[:, _SM["lg2"][0]:_SM["lg2"][1]]}
        eg34 = esm[:, _SM["lg3"][0]:_SM["lg4c"][1]]   # [128, 5]: lg3 | lg4c

        # s = eg * rsqrt(n2): magic seed + one Newton step.  The PSUM read
        # is on Vector; the chain arithmetic runs on GpSimd to keep the
        # Vector queue free for the batch-sized ops.
        gts = nc.gpsimd.tensor_scalar
        gstt = nc.gpsimd.scalar_tensor_tensor
        gmul = nc.gpsimd.tensor_mul

        def make_scale(n2_ap, eg_ap, shape, nm):
            pr = shape[0]
            n2s = scr.tile(list(shape), f32, name=f"n2s_{nm}", tag="sc_n2s")
            cp(n2s, n2_ap)
            shf = scr.tile(list(shape), u32, name=f"shf_{nm}", tag="sc_shf")
            ts(shf, n2s.bitcast(u32), 1, None, op0=ALU.arith_shift_right)
            y0 = scr.tile(list(shape), u32, name=f"y0_{nm}", tag="sc_y0")
            stt(y0, magict[:pr, :shape[1]], 0, shf, op0=ALU.bypass, op1=ALU.subtract)
            y = y0.bitcast(f32)
            t1 = scr.tile(list(shape), f32, name=f"t1_{nm}", tag="sc_t1")
            t2 = scr.tile(list(shape), f32, name=f"t2_{nm}", tag="sc_t2")
            gmul(t1, y, y)
            gmul(t2, t1, n2s)
            ts(t1, t2, -0.5, 1.5, op0=ALU.mult, op1=ALU.add)
            yn = scr.tile(list(shape), f32, name=f"yn_{nm}", tag="sc_yn")
            gmul(yn, y, t1)
            s = wgt.tile(list(shape), f32, name=f"s_{nm}", tag=f"s_{nm}")
            gmul(s, eg_ap, yn)
            return s

        # ---- v^2 pieces + direct columnized norm matmuls ------------------
        vsqO = {l: scr.tile([P, 4 * H], fp16, name=f"vsqO{l}", tag=f"vsqO{l}")
                for l in (2, 3)}
        vsqDt = scr.tile([P, 8 * P], fp16, name="vsqD", tag="vsqD")
        vsqD = {2: vsqDt[:, 0:4 * P], 3: vsqDt[:, 4 * P:8 * P]}

        def vsq_off(l, engs):
            for k in range(3):
                a, b = H * k + P * (k + 1), H * (k + 1)
                engs[k](vsqO[l][:, a:b], vtO[l][:, a:b], vtO[l][:, a:b],
                        op=ALU.mult)

        def vsq_diag():
            tt(vsqDt, vtDt, vtDt, op=ALU.mult)

        def ncol_direct(l):
            ncol = psN.tile([P, 4], f32, name=f"ncol{l}", tag="pn")
            # off-window contributions first (DMA-gated only), then diag.
            # Only the first-executed MM clears the bank (start=True): all
            # later MMs write-or-accumulate per element via has_written.
            for c in range(1, 4):
                for k in range(c):
                    mm(ncol[:, c:c + 1], vsqO[l][:, H * k + P * c:H * k + P * c + P],
                       ones2[:, 0:1], start=(c == 1 and k == 0), stop=False,
                       skip_group_check=True)
            for c in range(4):
                mm(ncol[:, c:c + 1], vsqD[l][:, P * c:P * c + P], ones2[:, 0:1],
                   start=False, stop=True, skip_group_check=True)
            return ncol

        # wdG: [dA 0; 0 dB] with input-side s_prev fold (per-partition)
        def make_wdG(l, s_prev):
            wdG = wgt.tile([P, 4 * P], fp16, name=f"wdG{l}", tag=f"wdG{l}")
            for c in range(4):
                ts(wdG[:, P * c:P * c + P], vtD[l][:, P * c:P * c + P],
                   s_prev[:, c:c + 1], None, op0=ALU.mult)
            gms(winap(wdG, 0, 64, 64, 4, P, 64), 0.0)   # zero the X quadrant
            return wdG

        # ================= layer 1 prep (natural layout [512,8]) ==========
        v1n = wgt.tile([P, 32], f32, name="v1n", tag="v1n")
        n1 = scr.tile([P, 4], f32, name="n1", tag="n1")
        vT1 = wgt.tile([8, H], fp16, name="vT1", tag="vT1")
        v1a = scr.tile([P, 32], f32, name="v1a", tag="v1a")
        mul(v1a, e1n, sm("md1n"))
        v1b = scr.tile([P, 32], f32, name="v1b", tag="v1b")
        mul(v1b, sm("w1n"), sm("mo1n"))
        tt(v1n, v1a, v1b, op=ALU.add)
        for c in range(4):
            sq1 = scr.tile([P, 8], f32, name=f"sq1_{c}", tag="sq1")
            stt(sq1, v1n[:, 8 * c:8 * c + 8], 0, v1n[:, 8 * c:8 * c + 8],
                op0=ALU.bypass, op1=ALU.mult, accum_out=n1[:, c:c + 1])
        s1 = make_scale(n1, eg[1], (P, 4), "l1")
        ptall = psN.tile([8, H], f32, name="ptall", tag="pn")
        for c in range(4):
            nc.tensor.transpose(ptall[:, P * c:P * c + P],
                                v1n[:, 8 * c:8 * c + 8], ident)
        cp(vT1, ptall)              # one PSUM->SBUF copy

        # L2 norm chain (s2) -- as early as possible
        vsq_off(2, (tt, gtt, gtt))  # first piece on Vector: earliest PE start
        vsq_diag()
        ncol2 = ncol_direct(2)
        s2 = make_scale(ncol2, eg[2], (P, 4), "l2")

        # ================= layer 1 batch ==================================
        h1 = wgt.tile([P, 4 * BC], fp16, name="h1", tag="h1")
        for c in range(4):
            z = pzc.tile([P, BC], f32, name=f"z1_{c}", tag="ps")
            mm(z, vT1[:, P * c:P * c + P], xT)
            act(h1[:, BC * c:BC * c + BC], z, E.Tanh,
                bias=sm("b1")[:, c:c + 1], scale=s1[:, c:c + 1])
        wdG2 = make_wdG(2, s1)
        # D1 = -F1 = (h1^2 - 1) * e1d  (sign flips cancel across layers)
        hq1 = scr.tile([P, 4 * BC], fp16, name="hq1", tag="hq1")
        tt(hq1, h1, h1, op=ALU.mult)
        D1 = wgt.tile([P, 4 * BC], bf16, name="D1", tag="D1")
        for c in range(4):
            # (hq * e1d) - e1d == e1d * (h^2 - 1)
            ts(D1[:, BC * c:BC * c + BC], hq1[:, BC * c:BC * c + BC],
               e1d[:, c:c + 1], e1d[:, c:c + 1], op0=ALU.mult, op1=ALU.subtract)

        # L3 norm pieces (start early; consumed mid-L2)
        vsq_off(3, (gtt, gtt, gtt))

        # ================= layer 2/3 batch ================================
        def big_batch(l, s, wdG, h_prev, D_prev, mid=None):
            hl = wgt.tile([P, 4 * BC], fp16, name=f"h{l}", tag=f"h{l}")
            zg = pzg.tile([P, 4 * BC], f32, name=f"zg{l}", tag="psg")
            for c in range(4):
                z = pzc.tile([P, BC], f32, name=f"zh{l}_{c}", tag="ps")
                for k in range(c + 1):
                    lhsT = (vtD[l][:, P * c:P * c + P] if k == c else
                            vtO[l][:, H * k + P * c:H * k + P * c + P])
                    mm(z, lhsT, h_prev[:, BC * k:BC * k + BC],
                       start=(k == 0), stop=(k == c))
                act(hl[:, BC * c:BC * c + BC], z, E.Tanh,
                    bias=sm(f"b{l}")[:, c:c + 1], scale=s[:, c:c + 1])
                mm(zg[:, BC * c:BC * c + BC], wdG[:, P * c:P * c + P],
                   D_prev[:, BC * c:BC * c + BC], start=True, stop=True)
                if mid is not None and c == 1:
                    mid()
            hql = scr.tile([P, 4 * BC], fp16, name=f"hq{l}", tag=f"hq{l}")
            if l == 3:
                # tail: per-chunk separate tiles so each pf4 matmul can
                # start as soon as its chunk's D is ready
                Dl = [wgt.tile([P, BC], bf16, name=f"D3_{c}", tag=f"D3_{c}")
                      for c in range(4)]
                for c in range(4):
                    cc = slice(BC * c, BC * c + BC)
                    tt(hql[:, cc], hl[:, cc], hl[:, cc], op=ALU.mult)
                    stt(Dl[c], hql[:, cc], 1.0, zg[:, cc],
                        op0=ALU.subtract, op1=ALU.mult)
            else:
                Dl = wgt.tile([P, 4 * BC], bf16, name=f"D{l}", tag=f"D{l}")
                tt(hql, hl, hl, op=ALU.mult)
                # D_l = (h^2 - 1) * zg  (zg = wdG @ D_{l-1}, PSUM fp32)
                stt(Dl, hql, 1.0, zg, op0=ALU.subtract, op1=ALU.mult)
            return hl, Dl

        # s3+s4 batched scale chain, emitted mid-L2 via callback
        s34_box = {}

        def mid_l2():
            ncol3 = ncol_direct(3)
            # layer-4 prep
            v4a = scr.tile([P, 32], f32, name="v4a", tag="v4a")
            mul(v4a, e4t, sm("md4t"))
            v4b = scr.tile([P, 32], f32, name="v4b", tag="v4b")
            mul(v4b, sm("w4t"), sm("mo4t"))
            vt4 = wgt.tile([P, 32], fp16, name="vt4", tag="vt4")
            tt(vt4, v4a, v4b, op=ALU.add)
            vsq4 = scr.tile([P, 32], fp16, name="vsq4", tag="vsq4")
            act(vsq4, vt4, E.Square)
            pn4 = psN.tile([8, 2], f32, name="pn4", tag="pn")
            for k in range(4):
                mm(pn4, vsq4[:, 8 * k:8 * k + 8], ones2,
                   start=(k == 0), stop=(k == 3))
            n34 = scr.tile([P, 5], f32, name="n34", tag="n34")
            nc.vector.memset(n34[:, 4:5], 1.0)   # rows 8.. of the s4 col
            cp(n34[:, 0:4], ncol3)
            cp(n34[0:8, 4:5], pn4[:, 0:1])
            s34 = make_scale(n34, eg34, (P, 5), "l34")
            s34_box["s3"] = s34[:, 0:4]
            s34_box["s4"] = s34[0:8, 4:5]
            s34_box["v4a"] = v4a
            s34_box["vt4"] = vt4

        h2, D2 = big_batch(2, s2, wdG2, h1, D1, mid=mid_l2)
        s3, s4 = s34_box["s3"], s34_box["s4"]
        v4a, vt4 = s34_box["v4a"], s34_box["vt4"]
        wdG3 = make_wdG(3, s2)

        h3, D3 = big_batch(3, s3, wdG3, h2, D2)

        vd4 = wgt.tile([P, 32], bf16, name="vd4", tag="vd4")
        for k in range(4):
            ts(vd4[:, 8 * k:8 * k + 8], v4a[:, 8 * k:8 * k + 8],
               s3[:, k:k + 1], None, op0=ALU.mult)

        # ================= layer 4 batch ==================================
        pz4 = pzc.tile([8, BC], f32, name="pz4", tag="ps")
        for k in range(4):
            mm(pz4, vt4[:, 8 * k:8 * k + 8], h3[:, BC * k:BC * k + BC],
               start=(k == 0), stop=(k == 3))
        h4 = wgt.tile([8, BC], f32, name="h4", tag="h4")
        act(h4, pz4, E.Tanh, bias=b4, scale=s4)
        nc.sync.dma_start(t["hT_out"], h4)
        pf4 = pzc.tile([8, BC], f32, name="pf4", tag="ps")
        for k in range(4):
            mm(pf4, vd4[:, 8 * k:8 * k + 8], D3[k],
               start=(k == 0), stop=(k == 3))
        hq4 = scr.tile([8, BC], f32, name="hq4", tag="hq4")
        mul(hq4, h4, h4)
        s24 = scr.tile([8, BC], f32, name="s24", tag="s24")
        ts(s24, hq4, 1.0, 1.0, op0=ALU.mult, op1=ALU.subtract)   # h4^2 - 1
        gt = scr.tile([8, BC], f32, name="gt", tag="gt")
        stt(gt, pf4, s4, s24, op0=ALU.mult, op1=ALU.mult)
        # fast natural log: ln(x) ~= A*bits(x) + B   (max abs err ~0.03)
        gbits = scr.tile([8, BC], f32, name="gbits", tag="gbits")
        cp(gbits, gt.bitcast(i32))
        sld = wgt.tile([8, BC], f32, name="sld", tag="sld")
        ts(sld, gbits, FASTLN_A, FASTLN_B, op0=ALU.mult, op1=ALU.add)
        nc.sync.dma_start(t["sldT_out"], sld)

    nc.compile()
    return nc


def _host_prep(x, W1, logg1, bias1, W2, logg2, bias2, W3, logg3, bias3,
               W4, logg4, bias4):
    """Pure layout prep (transpose / reshape / gather / masks / casts)."""
    f = np.float32

    def cols(a):          # [512]-ish vector -> [128, 4] column-chunk layout
        return np.ascontiguousarray(np.reshape(a, (4, P)).T).astype(f)

    smalls = np.zeros((P, SMALL_W), f)

    def put(name, arr):
        a, b = _SM[name]
        smalls[:arr.shape[0], a:b] = arr

    def fold(m):          # [512, 8] -> [128, (k x)] with k = row-chunk
        return m.reshape(4, P, 8).transpose(1, 0, 2).reshape(P, 32)

    put("ident", np.eye(P, dtype=f))
    put("w1n", fold(W1))                                   # natural [512,8]
    put("w4t", fold(np.ascontiguousarray(W4.T)))           # [512,8]
    put("w1dg", cols(W1[np.arange(H), np.arange(H) // 64]))
    put("w4dg", cols(W4[np.arange(H) // 64, np.arange(H)]))
    put("lg1", cols(logg1)); put("b1", cols(bias1))
    put("lg2", cols(logg2)); put("b2", cols(bias2))
    put("lg3", cols(logg3)); put("b3", cols(bias3))
    put("lg4c", logg4.reshape(8, 1).astype(f))
    put("b4c", bias4.reshape(8, 1).astype(f))
    # structural masks
    o = np.arange(H)[:, None] // 64
    i1 = np.arange(8)[None, :]
    md1 = (i1 == o).astype(f); mo1 = (i1 < o).astype(f)    # [512, 8] natural
    put("md1n", fold(md1)); put("mo1n", fold(mo1))
    ii = np.arange(H)[:, None] // 64
    o4 = np.arange(8)[None, :]
    md4 = (o4 == ii).astype(f); mo4 = (o4 > ii).astype(f)  # [512, 8] transposed
    put("md4t", fold(md4)); put("mo4t", fold(mo4))

    w2T = np.ascontiguousarray(W2.T).astype(np.float16)
    w3T = np.ascontiguousarray(W3.T).astype(np.float16)
    xT = np.ascontiguousarray(x.T).astype(np.float16)      # [8, 2048]
    return xT, w2T, w3T, smalls


def kernel(**inputs):
    global LAST_RESULTS
    from concourse.bass_utils import run_bass_kernel_spmd

    xT, w2T, w3T, smalls = _host_prep(**{k: np.asarray(v) for k, v in inputs.items()})

    if "nc" not in _CACHE:
        _CACHE["nc"] = _build()
    nc = _CACHE["nc"]

    in_maps = []
    for c in range(NCORE):
        in_maps.append({
            "xT": np.ascontiguousarray(xT[:, BC * c:BC * (c + 1)]),
            "w2T": w2T, "w3T": w3T, "smalls": smalls,
        })
    res = run_bass_kernel_spmd(nc, in_maps, core_ids=list(range(NCORE)),
                               trace=TRACE)
    LAST_RESULTS = res

    B = BC * NCORE
    h = np.empty((B, 8), np.float32)
    sld = np.empty((B, 8), np.float32)
    for c, r in enumerate(res.results):
        h[BC * c:BC * (c + 1)] = r["hT_out"].T
        sld[BC * c:BC * (c + 1)] = r["sldT_out"].T
    return h, sld


# revision 33
# speedup vs baseline: 1.1425x; 1.1425x over previous
"""BNAF forward (B=2048, D=8, H=512, 4 masked layers) on 8 TRN2 NeuronCores.

Strategy
--------
Pure data parallel: batch is split 256/core; the small weights are replicated.

Math: the BNAF log-det recursion collapses in exp space: exp(logdet diag
blocks) == diag blocks of the normalized weight, exp(tanh logdet) == 1-h^2,
so the flow is a chain of positive block-diag matmuls with one log at the
end (2-op DVE fast-log).  The norm scale s=exp(logg)/||v|| is folded
input-side into the next layer's G-flow weights (wd form).  The sech^2
factor is applied as (h^2-1) -- the sign flips cancel across the even
number of layers (with the matching (h4^2-1) fold at L4).

Tile layout notes (trn2 Tile framework tracks deps at TILE granularity, so
false-sharing serializes):
- per-(layer,chunk) PSUM tiles for the h-path matmuls so chunk c+1's MMs
  don't wait on chunk c's tanh (whole-tile WAR).
- weights live in vtO (full rows, DMA-only writers) + vtD (the 4 diag
  128-blocks, strided DMA) per layer, so norm/exp work on vtD never blocks
  reads of the raw off-diag blocks.
- norm^2 columnize is 10 direct lhsT=v^2-window matmuls (no row-sum /
  transpose machinery); one-step Newton rsqrt from a magic seed.
"""

import numpy as np

TRACE = False          # set by test.py for profiling runs
LAST_RESULTS = None    # BassKernelResults stash for test.py

_CACHE = {}

P = 128
BC = 256          # batch per core
H = 512
NCORE = 8
MAGIC = 0x5f3759df

# smalls layout: first the exp-batch block (one ACT op), then the rest
_SM = {}
_off = 0
for _name, _w in [("w1dg", 4), ("w4dg", 4), ("lg1", 4), ("lg2", 4),
                  ("lg3", 4), ("lg4c", 1), ("w1n", 32), ("w4t", 32),  # exp blk
                  ("b4c", 1), ("b1", 4), ("b2", 4), ("b3", 4),
                  ("ident", 128),
                  ("md1n", 32), ("mo1n", 32), ("md4t", 32), ("mo4t", 32)]:
    _SM[_name] = (_off, _off + _w)
    _off += _w
SMALL_W = _off
EXPW = _SM["w4t"][1]           # width of the exp block (85)

FASTLN_A = float(np.log(2.0) / (1 << 23))
FASTLN_B = float((0.0430 - 127.0) * np.log(2.0))


def _build():
    import concourse.bacc as bacc
    import concourse.mybir as mybir
    import concourse.tile as tile
    from concourse.bass import AP
    from contextlib import ExitStack

    f32 = mybir.dt.float32
    u32 = mybir.dt.uint32
    i32 = mybir.dt.int32
    bf16 = mybir.dt.bfloat16
    fp16 = mybir.dt.float16
    E = mybir.ActivationFunctionType
    ALU = mybir.AluOpType

    nc = bacc.Bacc("TRN2", target_bir_lowering=False, debug=False,
                   enable_asserts=False, num_devices=NCORE)

    t = {}
    t["xT"] = nc.dram_tensor("xT", (8, BC), fp16, kind="ExternalInput").ap()
    t["w2T"] = nc.dram_tensor("w2T", (H, H), fp16, kind="ExternalInput").ap()
    t["w3T"] = nc.dram_tensor("w3T", (H, H), fp16, kind="ExternalInput").ap()
    t["smalls"] = nc.dram_tensor("smalls", (P, SMALL_W), f32, kind="ExternalInput").ap()
    t["hT_out"] = nc.dram_tensor("hT_out", (8, BC), f32, kind="ExternalOutput").ap()
    t["sldT_out"] = nc.dram_tensor("sldT_out", (8, BC), f32, kind="ExternalOutput").ap()

    def mm(out, lhsT, rhs, **kw):
        nc.tensor.matmul(out, lhsT, rhs, **kw)

    def winap(base_tile, p0, np_, col0, n, stride, w):
        """[np_ parts at p0] x (n windows of w cols, stride apart, from col0)."""
        base = base_tile[p0:p0 + np_, col0:col0 + w]
        return AP(base.tensor, base.offset,
                  [[base.ap[0][0], np_], [stride, n], [1, w]])

    with tile.TileContext(nc) as tc, ExitStack() as ctx:
        wgt = ctx.enter_context(tc.tile_pool(name="wgt", bufs=1))
        scr = ctx.enter_context(tc.tile_pool(name="scr", bufs=4))
        psN = ctx.enter_context(tc.tile_pool(name="psN", bufs=2, space="PSUM"))
        pzc = ctx.enter_context(tc.tile_pool(name="pzc", bufs=4, space="PSUM"))
        pzg = ctx.enter_context(tc.tile_pool(name="pzg", bufs=1, space="PSUM"))

        act = nc.scalar.activation
        cp = nc.vector.tensor_copy
        ts = nc.vector.tensor_scalar
        stt = nc.vector.scalar_tensor_tensor
        mul = nc.vector.tensor_mul
        tt = nc.vector.tensor_tensor
        gtt = nc.gpsimd.tensor_tensor
        gms = nc.gpsimd.memset

        # ---- dummy ACT at t0 pulls the single exp_and_others table load ---
        dmy = wgt.tile([P, 1], f32, name="dmy", tag="dmy")
        dmyo = wgt.tile([P, 1], f32, name="dmyo", tag="dmyo")
        nc.vector.memset(dmy, 0.0)
        act(dmyo, dmy, E.Exp)
        # PE warmup on zeros: sustained activity through the DMA window so
        # the HAM clock gate opens (1.2 -> 2.4 GHz) before the real stream
        wz = wgt.tile([P, BC + 2], fp16, name="wz", tag="wz")
        pw = psN.tile([2, BC], f32, name="pw", tag="pn")
        nc.vector.memset(wz, 0.0)
        for _ in range(14):
            mm(pw, wz[:, 0:2], wz[:, 2:BC + 2], skip_group_check=True)

        # ---- input DMAs ---------------------------------------------------
        # vtO: full rows (raw W, DMA is the only writer).  vtD: the four
        # diag 128-blocks per layer, chunk c at cols [128c, 128c+128).
        smalls = wgt.tile([P, SMALL_W], f32, name="smalls_t", tag="smalls_t")
        xT = wgt.tile([8, BC], fp16, name="xT_t", tag="xT_t")
        vtO = {l: wgt.tile([P, 4 * H], fp16, name=f"vtO{l}", tag=f"vtO{l}")
               for l in (2, 3)}
        # one diag tile for both layers: block (l, c) at col 512(l-2)+128c
        vtDt = wgt.tile([P, 8 * P], fp16, name="vtD", tag="vtD")
        vtD = {2: vtDt[:, 0:4 * P], 3: vtDt[:, 4 * P:8 * P]}
        nc.sync.dma_start(smalls, t["smalls"])
        nc.gpsimd.dma_start(vtD[2], AP(t["w2T"].tensor, 0,
                                       [[H, P], [P * H + P, 4], [1, P]]))
        nc.gpsimd.dma_start(vtD[3], AP(t["w3T"].tensor, 0,
                                       [[H, P], [P * H + P, 4], [1, P]]))
        for hh in (0, 1):
            src = AP(t["w2T"].tensor, hh * 2 * P * H,
                     [[H, P], [P * H, 2], [1, H]])
            nc.sync.dma_start(vtO[2][:, 2 * H * hh:2 * H * hh + 2 * H], src)
        nc.sync.dma_start(xT, t["xT"])
        for hh in (0, 1):
            src = AP(t["w3T"].tensor, hh * 2 * P * H,
                     [[H, P], [P * H, 2], [1, H]])
            nc.gpsimd.dma_start(vtO[3][:, 2 * H * hh:2 * H * hh + 2 * H], src)

        def sm(name):
            a, b = _SM[name]
            return smalls[:, a:b]

        ident = sm("ident")
        b4 = smalls[0:8, _SM["b4c"][0]:_SM["b4c"][1]]

        # ---- constants ----------------------------------------------------
        ones2f = wgt.tile([P, 2], f32, name="ones2f", tag="ones2f")
        ones2 = wgt.tile([P, 2], fp16, name="ones2", tag="ones2")
        magict = wgt.tile([P, 5], u32, name="magict", tag="magict")
        gms(ones2f, 1.0)
        gms(magict, MAGIC)
        cp(ones2, ones2f)

        # ---- diag prep: zero LL quadrants, exp the 64-blocks (both layers)
        gms(winap(vtDt, 64, 64, 0, 8, P, 64), 0.0)
        dA = winap(vtDt, 0, 64, 0, 8, P, 64)
        dB = winap(vtDt, 64, 64, 64, 8, P, 64)
        act(dA, dA, E.Exp)
        act(dB, dB, E.Exp)

        # ---- one exp over all the small weight pieces ---------------------
        esm = wgt.tile([P, EXPW], f32, name="esm", tag="esm")
        act(esm, smalls[:, 0:EXPW], E.Exp)
        e1n = esm[:, _SM["w1n"][0]:_SM["w1n"][1]]
        e4t = esm[:, _SM["w4t"][0]:_SM["w4t"][1]]
        e1d = esm[:, _SM["w1dg"][0]:_SM["w1dg"][1]]
        eg = {1: esm[:, _SM["lg1"][0]:_SM["lg1"][1]],
              2: esm[:, _SM["lg2"][0]:_SM["lg2"][1]]}
        eg34 = esm[:, _SM["lg3"][0]:_SM["lg4c"][1]]   # [128, 5]: lg3 | lg4c

        # s = eg * rsqrt(n2): magic seed + one Newton step.  The PSUM read
        # is on Vector; the chain arithmetic runs on GpSimd to keep the
        # Vector queue free for the batch-sized ops.
        gts = nc.gpsimd.tensor_scalar
        gstt = nc.gpsimd.scalar_tensor_tensor
        gmul = nc.gpsimd.tensor_mul

        def make_scale(n2_ap, eg_ap, shape, nm):
            pr = shape[0]
            n2s = scr.tile(list(shape), f32, name=f"n2s_{nm}", tag="sc_n2s")
            cp(n2s, n2_ap)
            shf = scr.tile(list(shape), u32, name=f"shf_{nm}", tag="sc_shf")
            ts(shf, n2s.bitcast(u32), 1, None, op0=ALU.arith_shift_right)
            y0 = scr.tile(list(shape), u32, name=f"y0_{nm}", tag="sc_y0")
            stt(y0, magict[:pr, :shape[1]], 0, shf, op0=ALU.bypass, op1=ALU.subtract)
            y = y0.bitcast(f32)
            t1 = scr.tile(list(shape), f32, name=f"t1_{nm}", tag="sc_t1")
            t2 = scr.tile(list(shape), f32, name=f"t2_{nm}", tag="sc_t2")
            gmul(t1, y, y)
            gmul(t2, t1, n2s)
            ts(t1, t2, -0.5, 1.5, op0=ALU.mult, op1=ALU.add)
            yn = scr.tile(list(shape), f32, name=f"yn_{nm}", tag="sc_yn")
            gmul(yn, y, t1)
            s = wgt.tile(list(shape), f32, name=f"s_{nm}", tag=f"s_{nm}")
            gmul(s, eg_ap, yn)
            return s

        # ---- v^2 pieces + direct columnized norm matmuls ------------------
        vsqO = {l: scr.tile([P, 4 * H], fp16, name=f"vsqO{l}", tag=f"vsqO{l}")
                for l in (2, 3)}
        vsqDt = scr.tile([P, 8 * P], fp16, name="vsqD", tag="vsqD")
        vsqD = {2: vsqDt[:, 0:4 * P], 3: vsqDt[:, 4 * P:8 * P]}

        def vsq_off(l, engs):
            for k in range(3):
                a, b = H * k + P * (k + 1), H * (k + 1)
                engs[k](vsqO[l][:, a:b], vtO[l][:, a:b], vtO[l][:, a:b],
                        op=ALU.mult)

        def vsq_diag():
            tt(vsqDt, vtDt, vtDt, op=ALU.mult)

        def ncol_direct(l):
            ncol = psN.tile([P, 4], f32, name=f"ncol{l}", tag="pn")
            # off-window contributions first (DMA-gated only), then diag.
            # Only the first-executed MM clears the bank (start=True): all
            # later MMs write-or-accumulate per element via has_written.
            for c in range(1, 4):
                for k in range(c):
                    mm(ncol[:, c:c + 1], vsqO[l][:, H * k + P * c:H * k + P * c + P],
                       ones2[:, 0:1], start=(c == 1 and k == 0), stop=False,
                       skip_group_check=True)
            for c in range(4):
                mm(ncol[:, c:c + 1], vsqD[l][:, P * c:P * c + P], ones2[:, 0:1],
                   start=False, stop=True, skip_group_check=True)
            return ncol

        # wdG: [dA 0; 0 dB] with input-side s_prev fold (per-partition)
        def make_wdG(l, s_prev):
            wdG = wgt.tile([P, 4 * P], fp16, name=f"wdG{l}", tag=f"wdG{l}")
            for c in range(4):
                ts(wdG[:, P * c:P * c + P], vtD[l][:, P * c:P * c + P],
                   s_prev[:, c:c + 1], None, op0=ALU.mult)
            gms(winap(wdG, 0, 64, 64, 4, P, 64), 0.0)   # zero the X quadrant
            return wdG

        # ================= layer 1 prep (natural layout [512,8]) ==========
        v1n = wgt.tile([P, 32], f32, name="v1n", tag="v1n")
        n1 = scr.tile([P, 4], f32, name="n1", tag="n1")
        vT1 = wgt.tile([8, H], fp16, name="vT1", tag="vT1")
        v1a = scr.tile([P, 32], f32, name="v1a", tag="v1a")
        mul(v1a, e1n, sm("md1n"))
        v1b = scr.tile([P, 32], f32, name="v1b", tag="v1b")
        mul(v1b, sm("w1n"), sm("mo1n"))
        tt(v1n, v1a, v1b, op=ALU.add)
        for c in range(4):
            sq1 = scr.tile([P, 8], f32, name=f"sq1_{c}", tag="sq1")
            stt(sq1, v1n[:, 8 * c:8 * c + 8], 0, v1n[:, 8 * c:8 * c + 8],
                op0=ALU.bypass, op1=ALU.mult, accum_out=n1[:, c:c + 1])
        s1 = make_scale(n1, eg[1], (P, 4), "l1")
        ptall = psN.tile([8, H], f32, name="ptall", tag="pn")
        for c in range(4):
            nc.tensor.transpose(ptall[:, P * c:P * c + P],
                                v1n[:, 8 * c:8 * c + 8], ident)
        cp(vT1, ptall)              # one PSUM->SBUF copy

        # L2 norm chain (s2) -- as early as possible
        vsq_off(2, (tt, gtt, gtt))  # first piece on Vector: earliest PE start
        vsq_diag()
        ncol2 = ncol_direct(2)
        s2 = make_scale(ncol2, eg[2], (P, 4), "l2")

        # ================= layer 1 batch ==================================
        h1 = wgt.tile([P, 4 * BC], fp16, name="h1", tag="h1")
        for c in range(4):
            z = pzc.tile([P, BC], f32, name=f"z1_{c}", tag="ps")
            mm(z, vT1[:, P * c:P * c + P], xT)
            act(h1[:, BC * c:BC * c + BC], z, E.Tanh,
                bias=sm("b1")[:, c:c + 1], scale=s1[:, c:c + 1])
        wdG2 = make_wdG(2, s1)
        # D1 = -F1 = (h1^2 - 1) * e1d  (sign flips cancel across layers)
        hq1 = scr.tile([P, 4 * BC], fp16, name="hq1", tag="hq1")
        tt(hq1, h1, h1, op=ALU.mult)
        D1 = wgt.tile([P, 4 * BC], bf16, name="D1", tag="D1")
        for c in range(4):
            # (hq * e1d) - e1d == e1d * (h^2 - 1)
            ts(D1[:, BC * c:BC * c + BC], hq1[:, BC * c:BC * c + BC],
               e1d[:, c:c + 1], e1d[:, c:c + 1], op0=ALU.mult, op1=ALU.subtract)

        # L3 norm pieces (start early; consumed mid-L2)
        vsq_off(3, (gtt, gtt, gtt))

        # ================= layer 2/3 batch ================================
        def big_batch(l, s, wdG, h_prev, D_prev, mid=None):
            hl = wgt.tile([P, 4 * BC], fp16, name=f"h{l}", tag=f"h{l}")
            zg = pzg.tile([P, 4 * BC], f32, name=f"zg{l}", tag="psg")
            for c in range(4):
                z = pzc.tile([P, BC], f32, name=f"zh{l}_{c}", tag="ps")
                for k in range(c + 1):
                    lhsT = (vtD[l][:, P * c:P * c + P] if k == c else
                            vtO[l][:, H * k + P * c:H * k + P * c + P])
                    mm(z, lhsT, h_prev[:, BC * k:BC * k + BC],
                       start=(k == 0), stop=(k == c))
                act(hl[:, BC * c:BC * c + BC], z, E.Tanh,
                    bias=sm(f"b{l}")[:, c:c + 1], scale=s[:, c:c + 1])
                mm(zg[:, BC * c:BC * c + BC], wdG[:, P * c:P * c + P],
                   D_prev[:, BC * c:BC * c + BC], start=True, stop=True)
                if mid is not None and c == 1:
                    mid()
            hql = scr.tile([P, 4 * BC], fp16, name=f"hq{l}", tag=f"hq{l}")
            Dl = wgt.tile([P, 4 * BC], bf16, name=f"D{l}", tag=f"D{l}")
            if l == 2:
                tt(hql, hl, hl, op=ALU.mult)
            else:
                act(hql, hl, E.Square)
            # D_l = (h^2 - 1) * zg  (zg = wdG @ D_{l-1}, PSUM fp32)
            stt(Dl, hql, 1.0, zg, op0=ALU.subtract, op1=ALU.mult)
            return hl, Dl

        # s3+s4 batched scale chain, emitted mid-L2 via callback
        s34_box = {}

        def mid_l2():
            ncol3 = ncol_direct(3)
            # layer-4 prep
            v4a = scr.tile([P, 32], f32, name="v4a", tag="v4a")
            mul(v4a, e4t, sm("md4t"))
            v4b = scr.tile([P, 32], f32, name="v4b", tag="v4b")
            mul(v4b, sm("w4t"), sm("mo4t"))
            vt4 = wgt.tile([P, 32], fp16, name="vt4", tag="vt4")
            tt(vt4, v4a, v4b, op=ALU.add)
            vsq4 = scr.tile([P, 32], fp16, name="vsq4", tag="vsq4")
            act(vsq4, vt4, E.Square)
            pn4 = psN.tile([8, 2], f32, name="pn4", tag="pn")
            for k in range(4):
                mm(pn4, vsq4[:, 8 * k:8 * k + 8], ones2,
                   start=(k == 0), stop=(k == 3))
            n34 = scr.tile([P, 5], f32, name="n34", tag="n34")
            nc.vector.memset(n34[:, 4:5], 1.0)   # rows 8.. of the s4 col
            cp(n34[:, 0:4], ncol3)
            cp(n34[0:8, 4:5], pn4[:, 0:1])
            s34 = make_scale(n34, eg34, (P, 5), "l34")
            s34_box["s3"] = s34[:, 0:4]
            s34_box["s4"] = s34[0:8, 4:5]
            s34_box["v4a"] = v4a
            s34_box["vt4"] = vt4

        h2, D2 = big_batch(2, s2, wdG2, h1, D1, mid=mid_l2)
        s3, s4 = s34_box["s3"], s34_box["s4"]
        v4a, vt4 = s34_box["v4a"], s34_box["vt4"]
        wdG3 = make_wdG(3, s2)

        h3, D3 = big_batch(3, s3, wdG3, h2, D2)

        vd4 = wgt.tile([P, 32], bf16, name="vd4", tag="vd4")
        for k in range(4):
            ts(vd4[:, 8 * k:8 * k + 8], v4a[:, 8 * k:8 * k + 8],
               s3[:, k:k + 1], None, op0=ALU.mult)

        # ================= layer 4 batch ==================================
        pz4 = pzc.tile([8, BC], f32, name="pz4", tag="ps")
        for k in range(4):
            mm(pz4, vt4[:, 8 * k:8 * k + 8], h3[:, BC * k:BC * k + BC],
               start=(k == 0), stop=(k == 3))
        h4 = wgt.tile([8, BC], f32, name="h4", tag="h4")
        act(h4, pz4, E.Tanh, bias=b4, scale=s4)
        nc.sync.dma_start(t["hT_out"], h4)
        pf4 = pzc.tile([8, BC], f32, name="pf4", tag="ps")
        for k in range(4):
            mm(pf4, vd4[:, 8 * k:8 * k + 8], D3[:, BC * k:BC * k + BC],
               start=(k == 0), stop=(k == 3))
        hq4 = scr.tile([8, BC], f32, name="hq4", tag="hq4")
        mul(hq4, h4, h4)
        s24 = scr.tile([8, BC], f32, name="s24", tag="s24")
        ts(s24, hq4, 1.0, 1.0, op0=ALU.mult, op1=ALU.subtract)   # h4^2 - 1
        gt = scr.tile([8, BC], f32, name="gt", tag="gt")
        stt(gt, pf4, s4, s24, op0=ALU.mult, op1=ALU.mult)
        # fast natural log: ln(x) ~= A*bits(x) + B   (max abs err ~0.03)
        gbits = scr.tile([8, BC], f32, name="gbits", tag="gbits")
        cp(gbits, gt.bitcast(i32))
        sld = wgt.tile([8, BC], f32, name="sld", tag="sld")
        ts(sld, gbits, FASTLN_A, FASTLN_B, op0=ALU.mult, op1=ALU.add)
        nc.sync.dma_start(t["sldT_out"], sld)

    nc.compile()
    return nc


def _host_prep(x, W1, logg1, bias1, W2, logg2, bias2, W3, logg3, bias3,
               W4, logg4, bias4):
    """Pure layout prep (transpose / reshape / gather / masks / casts)."""
    f = np.float32

    def cols(a):          # [512]-ish vector -> [128, 4] column-chunk layout
        return np.ascontiguousarray(np.reshape(a, (4, P)).T).astype(f)

    smalls = np.zeros((P, SMALL_W), f)

    def put(name, arr):
        a, b = _SM[name]
        smalls[:arr.shape[0], a:b] = arr

    def fold(m):          # [512, 8] -> [128, (k x)] with k = row-chunk
        return m.reshape(4, P, 8).transpose(1, 0, 2).reshape(P, 32)

    put("ident", np.eye(P, dtype=f))
    put("w1n", fold(W1))                                   # natural [512,8]
    put("w4t", fold(np.ascontiguousarray(W4.T)))           # [512,8]
    put("w1dg", cols(W1[np.arange(H), np.arange(H) // 64]))
    put("w4dg", cols(W4[np.arange(H) // 64, np.arange(H)]))
    put("lg1", cols(logg1)); put("b1", cols(bias1))
    put("lg2", cols(logg2)); put("b2", cols(bias2))
    put("lg3", cols(logg3)); put("b3", cols(bias3))
    put("lg4c", logg4.reshape(8, 1).astype(f))
    put("b4c", bias4.reshape(8, 1).astype(f))
    # structural masks
    o = np.arange(H)[:, None] // 64
    i1 = np.arange(8)[None, :]
    md1 = (i1 == o).astype(f); mo1 = (i1 < o).astype(f)    # [512, 8] natural
    put("md1n", fold(md1)); put("mo1n", fold(mo1))
    ii = np.arange(H)[:, None] // 64
    o4 = np.arange(8)[None, :]
    md4 = (o4 == ii).astype(f); mo4 = (o4 > ii).astype(f)  # [512, 8] transposed
    put("md4t", fold(md4)); put("mo4t", fold(mo4))

    w2T = np.ascontiguousarray(W2.T).astype(np.float16)
    w3T = np.ascontiguousarray(W3.T).astype(np.float16)
    xT = np.ascontiguousarray(x.T).astype(np.float16)      # [8, 2048]
    return xT, w2T, w3T, smalls


def kernel(**inputs):
    global LAST_RESULTS
    from concourse.bass_utils import run_bass_kernel_spmd

    xT, w2T, w3T, smalls = _host_prep(**{k: np.asarray(v) for k, v in inputs.items()})

    if "nc" not in _CACHE:
        _CACHE["nc"] = _build()
    nc = _CACHE["nc"]

    in_maps = []
    for c in range(NCORE):
        in_maps.append({
            "xT": np.ascontiguousarray(xT[:, BC * c:BC * (c + 1)]),
            "w2T": w2T, "w3T": w3T, "smalls": smalls,
        })
    res = run_bass_kernel_spmd(nc, in_maps, core_ids=list(range(NCORE)),
                               trace=TRACE)
    LAST_RESULTS = res

    B = BC * NCORE
    h = np.empty((B, 8), np.float32)
    sld = np.empty((B, 8), np.float32)
    for c, r in enumerate(res.results):
        h[BC * c:BC * (c + 1)] = r["hT_out"].T
        sld[BC * c:BC * (c + 1)] = r["sldT_out"].T
    return h, sld


# revision 36
# speedup vs baseline: 1.1759x; 1.0292x over previous
"""BNAF forward (B=2048, D=8, H=512, 4 masked layers) on 8 TRN2 NeuronCores.

Strategy
--------
Pure data parallel: batch is split 256/core; the small weights are replicated.

Math: the BNAF log-det recursion collapses in exp space: exp(logdet diag
blocks) == diag blocks of the normalized weight, exp(tanh logdet) == 1-h^2,
so the flow is a chain of positive block-diag matmuls with one log at the
end (2-op DVE fast-log).  The norm scale s=exp(logg)/||v|| is folded
input-side into the next layer's G-flow weights (wd form).  The sech^2
factor is applied as (h^2-1) -- the sign flips cancel across the even
number of layers (with the matching (h4^2-1) fold at L4).

Tile layout notes (trn2 Tile framework tracks deps at TILE granularity, so
false-sharing serializes):
- per-(layer,chunk) PSUM tiles for the h-path matmuls so chunk c+1's MMs
  don't wait on chunk c's tanh (whole-tile WAR).
- weights live in vtO (full rows, DMA-only writers) + vtD (the 4 diag
  128-blocks, strided DMA) per layer, so norm/exp work on vtD never blocks
  reads of the raw off-diag blocks.
- norm^2 columnize is 10 direct lhsT=v^2-window matmuls (no row-sum /
  transpose machinery); one-step Newton rsqrt from a magic seed.
"""

import numpy as np

TRACE = False          # set by test.py for profiling runs
LAST_RESULTS = None    # BassKernelResults stash for test.py

_CACHE = {}

P = 128
BC = 256          # batch per core
H = 512
NCORE = 8
MAGIC = 0x5f3759df

# smalls layout: first the exp-batch block (one ACT op), then the rest
_SM = {}
_off = 0
for _name, _w in [("w1dg", 4), ("w4dg", 4), ("lg1", 4), ("lg2", 4),
                  ("lg3", 4), ("lg4c", 1), ("w1n", 32), ("w4t", 32),  # exp blk
                  ("b4c", 1), ("b1", 4), ("b2", 4), ("b3", 4),
                  ("ident", 128),
                  ("md1n", 32), ("mo1n", 32), ("md4t", 32), ("mo4t", 32)]:
    _SM[_name] = (_off, _off + _w)
    _off += _w
SMALL_W = _off
EXPW = _SM["w4t"][1]           # width of the exp block (85)

FASTLN_A = float(np.log(2.0) / (1 << 23))
FASTLN_B = float((0.0430 - 127.0) * np.log(2.0))


def _build():
    import concourse.bacc as bacc
    import concourse.mybir as mybir
    import concourse.tile as tile
    from concourse.bass import AP
    from contextlib import ExitStack

    f32 = mybir.dt.float32
    u32 = mybir.dt.uint32
    i32 = mybir.dt.int32
    bf16 = mybir.dt.bfloat16
    fp16 = mybir.dt.float16
    E = mybir.ActivationFunctionType
    ALU = mybir.AluOpType

    nc = bacc.Bacc("TRN2", target_bir_lowering=False, debug=False,
                   enable_asserts=False, num_devices=NCORE)

    t = {}
    t["xT"] = nc.dram_tensor("xT", (8, BC), fp16, kind="ExternalInput").ap()
    t["w2T"] = nc.dram_tensor("w2T", (H, H), fp16, kind="ExternalInput").ap()
    t["w3T"] = nc.dram_tensor("w3T", (H, H), fp16, kind="ExternalInput").ap()
    t["smalls"] = nc.dram_tensor("smalls", (P, SMALL_W), f32, kind="ExternalInput").ap()
    t["hT_out"] = nc.dram_tensor("hT_out", (8, BC), f32, kind="ExternalOutput").ap()
    t["sldT_out"] = nc.dram_tensor("sldT_out", (8, BC), f32, kind="ExternalOutput").ap()

    def mm(out, lhsT, rhs, **kw):
        nc.tensor.matmul(out, lhsT, rhs, **kw)

    def winap(base_tile, p0, np_, col0, n, stride, w):
        """[np_ parts at p0] x (n windows of w cols, stride apart, from col0)."""
        base = base_tile[p0:p0 + np_, col0:col0 + w]
        return AP(base.tensor, base.offset,
                  [[base.ap[0][0], np_], [stride, n], [1, w]])

    with tile.TileContext(nc) as tc, ExitStack() as ctx:
        wgt = ctx.enter_context(tc.tile_pool(name="wgt", bufs=1))
        scr = ctx.enter_context(tc.tile_pool(name="scr", bufs=4))
        psN = ctx.enter_context(tc.tile_pool(name="psN", bufs=2, space="PSUM"))
        pzc = ctx.enter_context(tc.tile_pool(name="pzc", bufs=4, space="PSUM"))
        pzg = ctx.enter_context(tc.tile_pool(name="pzg", bufs=1, space="PSUM"))

        act = nc.scalar.activation
        cp = nc.vector.tensor_copy
        ts = nc.vector.tensor_scalar
        stt = nc.vector.scalar_tensor_tensor
        mul = nc.vector.tensor_mul
        tt = nc.vector.tensor_tensor
        gtt = nc.gpsimd.tensor_tensor
        gms = nc.gpsimd.memset

        # ---- dummy ACT at t0 pulls the single exp_and_others table load ---
        dmy = wgt.tile([P, 1], f32, name="dmy", tag="dmy")
        dmyo = wgt.tile([P, 1], f32, name="dmyo", tag="dmyo")
        nc.vector.memset(dmy, 0.0)
        act(dmyo, dmy, E.Exp)
        # PE warmup on zeros: sustained activity through the DMA window so
        # the HAM clock gate opens (1.2 -> 2.4 GHz) before the real stream
        wz = wgt.tile([P, BC + 2], fp16, name="wz", tag="wz")
        pw = psN.tile([2, BC], f32, name="pw", tag="pn")
        nc.vector.memset(wz, 0.0)
        for _ in range(14):
            mm(pw, wz[:, 0:2], wz[:, 2:BC + 2], skip_group_check=True)

        # ---- input DMAs ---------------------------------------------------
        # vtO: full rows (raw W, DMA is the only writer).  vtD: the four
        # diag 128-blocks per layer, chunk c at cols [128c, 128c+128).
        smalls = wgt.tile([P, SMALL_W], f32, name="smalls_t", tag="smalls_t")
        xT = wgt.tile([8, BC], fp16, name="xT_t", tag="xT_t")
        vtO = {l: wgt.tile([P, 4 * H], fp16, name=f"vtO{l}", tag=f"vtO{l}")
               for l in (2, 3)}
        # one diag tile for both layers: block (l, c) at col 512(l-2)+128c
        vtDt = wgt.tile([P, 8 * P], fp16, name="vtD", tag="vtD")
        vtD = {2: vtDt[:, 0:4 * P], 3: vtDt[:, 4 * P:8 * P]}
        nc.sync.dma_start(smalls, t["smalls"])
        nc.gpsimd.dma_start(vtD[2], AP(t["w2T"].tensor, 0,
                                       [[H, P], [P * H + P, 4], [1, P]]))
        nc.gpsimd.dma_start(vtD[3], AP(t["w3T"].tensor, 0,
                                       [[H, P], [P * H + P, 4], [1, P]]))
        for hh in (0, 1):
            src = AP(t["w2T"].tensor, hh * 2 * P * H,
                     [[H, P], [P * H, 2], [1, H]])
            nc.sync.dma_start(vtO[2][:, 2 * H * hh:2 * H * hh + 2 * H], src)
        nc.sync.dma_start(xT, t["xT"])
        for hh in (0, 1):
            src = AP(t["w3T"].tensor, hh * 2 * P * H,
                     [[H, P], [P * H, 2], [1, H]])
            nc.gpsimd.dma_start(vtO[3][:, 2 * H * hh:2 * H * hh + 2 * H], src)

        def sm(name):
            a, b = _SM[name]
            return smalls[:, a:b]

        ident = sm("ident")
        b4 = smalls[0:8, _SM["b4c"][0]:_SM["b4c"][1]]

        # ---- constants ----------------------------------------------------
        ones2f = wgt.tile([P, 2], f32, name="ones2f", tag="ones2f")
        ones2 = wgt.tile([P, 2], fp16, name="ones2", tag="ones2")
        magict = wgt.tile([P, 5], u32, name="magict", tag="magict")
        gms(ones2f, 1.0)
        gms(magict, MAGIC)
        cp(ones2, ones2f)

        # ---- diag prep: zero LL quadrants, exp the 64-blocks (both layers)
        gms(winap(vtDt, 64, 64, 0, 8, P, 64), 0.0)
        dA = winap(vtDt, 0, 64, 0, 8, P, 64)
        dB = winap(vtDt, 64, 64, 64, 8, P, 64)
        act(dA, dA, E.Exp)
        act(dB, dB, E.Exp)

        # ---- one exp over all the small weight pieces ---------------------
        esm = wgt.tile([P, EXPW], f32, name="esm", tag="esm")
        act(esm, smalls[:, 0:EXPW], E.Exp)
        e1n = esm[:, _SM["w1n"][0]:_SM["w1n"][1]]
        e4t = esm[:, _SM["w4t"][0]:_SM["w4t"][1]]
        e1d = esm[:, _SM["w1dg"][0]:_SM["w1dg"][1]]
        eg = {1: esm[:, _SM["lg1"][0]:_SM["lg1"][1]],
              2: esm[:, _SM["lg2"][0]:_SM["lg2"][1]]}
        eg34 = esm[:, _SM["lg3"][0]:_SM["lg4c"][1]]   # [128, 5]: lg3 | lg4c

        # s = eg * rsqrt(n2): magic seed + one Newton step.  The PSUM read
        # is on Vector; the chain arithmetic runs on GpSimd to keep the
        # Vector queue free for the batch-sized ops.
        gts = nc.gpsimd.tensor_scalar
        gstt = nc.gpsimd.scalar_tensor_tensor
        gmul = nc.gpsimd.tensor_mul

        def make_scale(n2_ap, eg_ap, shape, nm, xmul=mul, xts=None, xstt=None):
            xts = xts or ts
            xstt = xstt or stt
            pr = shape[0]
            n2s = scr.tile(list(shape), f32, name=f"n2s_{nm}", tag="sc_n2s")
            cp(n2s, n2_ap)
            shf = scr.tile(list(shape), u32, name=f"shf_{nm}", tag="sc_shf")
            xts(shf, n2s.bitcast(u32), 1, None, op0=ALU.arith_shift_right)
            y0 = scr.tile(list(shape), u32, name=f"y0_{nm}", tag="sc_y0")
            xstt(y0, magict[:pr, :shape[1]], 0, shf, op0=ALU.bypass, op1=ALU.subtract)
            y = y0.bitcast(f32)
            t1 = scr.tile(list(shape), f32, name=f"t1_{nm}", tag="sc_t1")
            t2 = scr.tile(list(shape), f32, name=f"t2_{nm}", tag="sc_t2")
            xmul(t1, y, y)
            xmul(t2, t1, n2s)
            xts(t1, t2, -0.5, 1.5, op0=ALU.mult, op1=ALU.add)
            yn = scr.tile(list(shape), f32, name=f"yn_{nm}", tag="sc_yn")
            xmul(yn, y, t1)
            s = wgt.tile(list(shape), f32, name=f"s_{nm}", tag=f"s_{nm}")
            xmul(s, eg_ap, yn)
            return s

        # ---- v^2 pieces + direct columnized norm matmuls ------------------
        vsqO = {l: scr.tile([P, 4 * H], fp16, name=f"vsqO{l}", tag=f"vsqO{l}")
                for l in (2, 3)}
        vsqDt = scr.tile([P, 8 * P], fp16, name="vsqD", tag="vsqD")
        vsqD = {2: vsqDt[:, 0:4 * P], 3: vsqDt[:, 4 * P:8 * P]}

        def vsq_off(l, engs):
            for k in range(3):
                a, b = H * k + P * (k + 1), H * (k + 1)
                engs[k](vsqO[l][:, a:b], vtO[l][:, a:b], vtO[l][:, a:b],
                        op=ALU.mult)

        def vsq_diag():
            tt(vsqDt, vtDt, vtDt, op=ALU.mult)

        def ncol_direct(l):
            ncol = psN.tile([P, 4], f32, name=f"ncol{l}", tag="pn")
            # off-window contributions first (DMA-gated only), then diag.
            # Only the first-executed MM clears the bank (start=True): all
            # later MMs write-or-accumulate per element via has_written.
            for c in range(1, 4):
                for k in range(c):
                    mm(ncol[:, c:c + 1], vsqO[l][:, H * k + P * c:H * k + P * c + P],
                       ones2[:, 0:1], start=(c == 1 and k == 0), stop=False,
                       skip_group_check=True)
            for c in range(4):
                mm(ncol[:, c:c + 1], vsqD[l][:, P * c:P * c + P], ones2[:, 0:1],
                   start=False, stop=True, skip_group_check=True)
            return ncol

        # wdG: [dA 0; 0 dB] with input-side s_prev fold (per-partition)
        def make_wdG(l, s_prev):
            wdG = wgt.tile([P, 4 * P], fp16, name=f"wdG{l}", tag=f"wdG{l}")
            for c in range(4):
                ts(wdG[:, P * c:P * c + P], vtD[l][:, P * c:P * c + P],
                   s_prev[:, c:c + 1], None, op0=ALU.mult)
            gms(winap(wdG, 0, 64, 64, 4, P, 64), 0.0)   # zero the X quadrant
            return wdG

        # ================= layer 1 prep (natural layout [512,8]) ==========
        v1n = wgt.tile([P, 32], f32, name="v1n", tag="v1n")
        n1 = scr.tile([P, 4], f32, name="n1", tag="n1")
        vT1 = wgt.tile([8, H], fp16, name="vT1", tag="vT1")
        v1a = scr.tile([P, 32], f32, name="v1a", tag="v1a")
        mul(v1a, e1n, sm("md1n"))
        v1b = scr.tile([P, 32], f32, name="v1b", tag="v1b")
        mul(v1b, sm("w1n"), sm("mo1n"))
        tt(v1n, v1a, v1b, op=ALU.add)
        for c in range(4):
            sq1 = scr.tile([P, 8], f32, name=f"sq1_{c}", tag="sq1")
            stt(sq1, v1n[:, 8 * c:8 * c + 8], 0, v1n[:, 8 * c:8 * c + 8],
                op0=ALU.bypass, op1=ALU.mult, accum_out=n1[:, c:c + 1])
        s1 = make_scale(n1, eg[1], (P, 4), "l1")
        ptall = psN.tile([8, H], f32, name="ptall", tag="pn")
        for c in range(4):
            nc.tensor.transpose(ptall[:, P * c:P * c + P],
                                v1n[:, 8 * c:8 * c + 8], ident)
        act(vT1, ptall, E.Copy)     # one PSUM->SBUF copy, on idle Scalar

        # L2 norm chain (s2) -- as early as possible
        vsq_off(2, (tt, gtt, gtt))  # first piece on Vector: earliest PE start
        vsq_diag()
        ncol2 = ncol_direct(2)
        s2 = make_scale(ncol2, eg[2], (P, 4), "l2")

        # ================= layer 1 batch ==================================
        h1 = wgt.tile([P, 4 * BC], fp16, name="h1", tag="h1")
        for c in range(4):
            z = pzc.tile([P, BC], f32, name=f"z1_{c}", tag="ps")
            mm(z, vT1[:, P * c:P * c + P], xT)
            act(h1[:, BC * c:BC * c + BC], z, E.Tanh,
                bias=sm("b1")[:, c:c + 1], scale=s1[:, c:c + 1])
        wdG2 = make_wdG(2, s1)
        # D1 = -F1 = (h1^2 - 1) * e1d  (sign flips cancel across layers)
        hq1 = scr.tile([P, 4 * BC], fp16, name="hq1", tag="hq1")
        tt(hq1, h1, h1, op=ALU.mult)
        D1 = wgt.tile([P, 4 * BC], bf16, name="D1", tag="D1")
        for c in range(4):
            # (hq * e1d) - e1d == e1d * (h^2 - 1)
            ts(D1[:, BC * c:BC * c + BC], hq1[:, BC * c:BC * c + BC],
               e1d[:, c:c + 1], e1d[:, c:c + 1], op0=ALU.mult, op1=ALU.subtract)

        # L3 norm pieces (start early; consumed mid-L2)
        vsq_off(3, (gtt, gtt, gtt))

        # ================= layer 2/3 batch ================================
        def big_batch(l, s, wdG, h_prev, D_prev, mid=None):
            hl = wgt.tile([P, 4 * BC], fp16, name=f"h{l}", tag=f"h{l}")
            zg = pzg.tile([P, 4 * BC], f32, name=f"zg{l}", tag="psg")
            for c in range(4):
                z = pzc.tile([P, BC], f32, name=f"zh{l}_{c}", tag="ps")
                for k in range(c + 1):
                    lhsT = (vtD[l][:, P * c:P * c + P] if k == c else
                            vtO[l][:, H * k + P * c:H * k + P * c + P])
                    mm(z, lhsT, h_prev[:, BC * k:BC * k + BC],
                       start=(k == 0), stop=(k == c))
                act(hl[:, BC * c:BC * c + BC], z, E.Tanh,
                    bias=sm(f"b{l}")[:, c:c + 1], scale=s[:, c:c + 1])
                mm(zg[:, BC * c:BC * c + BC], wdG[:, P * c:P * c + P],
                   D_prev[:, BC * c:BC * c + BC], start=True, stop=True)
                if mid is not None and c == 1:
                    mid()
            hql = scr.tile([P, 4 * BC], fp16, name=f"hq{l}", tag=f"hq{l}")
            Dl = wgt.tile([P, 4 * BC], bf16, name=f"D{l}", tag=f"D{l}")
            if l == 2:
                tt(hql, hl, hl, op=ALU.mult)
            else:
                act(hql, hl, E.Square)
            # D_l = (h^2 - 1) * zg  (zg = wdG @ D_{l-1}, PSUM fp32)
            stt(Dl, hql, 1.0, zg, op0=ALU.subtract, op1=ALU.mult)
            return hl, Dl

        # s3+s4 batched scale chain, emitted mid-L2 via callback
        s34_box = {}

        def mid_l2():
            ncol3 = ncol_direct(3)
            # layer-4 prep
            v4a = scr.tile([P, 32], f32, name="v4a", tag="v4a")
            mul(v4a, e4t, sm("md4t"))
            v4b = scr.tile([P, 32], f32, name="v4b", tag="v4b")
            mul(v4b, sm("w4t"), sm("mo4t"))
            vt4 = wgt.tile([P, 32], fp16, name="vt4", tag="vt4")
            tt(vt4, v4a, v4b, op=ALU.add)
            vsq4 = scr.tile([P, 32], fp16, name="vsq4", tag="vsq4")
            act(vsq4, vt4, E.Square)
            pn4 = psN.tile([8, 2], f32, name="pn4", tag="pn")
            for k in range(4):
                mm(pn4, vsq4[:, 8 * k:8 * k + 8], ones2,
                   start=(k == 0), stop=(k == 3))
            n34 = scr.tile([P, 5], f32, name="n34", tag="n34")
            nc.vector.memset(n34[:, 4:5], 1.0)   # rows 8.. of the s4 col
            cp(n34[:, 0:4], ncol3)
            cp(n34[0:8, 4:5], pn4[:, 0:1])
            s34 = make_scale(n34, eg34, (P, 5), "l34", xmul=gmul)
            s34_box["s3"] = s34[:, 0:4]
            s34_box["s4"] = s34[0:8, 4:5]
            s34_box["v4a"] = v4a
            s34_box["vt4"] = vt4

        h2, D2 = big_batch(2, s2, wdG2, h1, D1, mid=mid_l2)
        s3, s4 = s34_box["s3"], s34_box["s4"]
        v4a, vt4 = s34_box["v4a"], s34_box["vt4"]
        wdG3 = make_wdG(3, s2)

        h3, D3 = big_batch(3, s3, wdG3, h2, D2)

        vd4 = wgt.tile([P, 32], bf16, name="vd4", tag="vd4")
        for k in range(4):
            ts(vd4[:, 8 * k:8 * k + 8], v4a[:, 8 * k:8 * k + 8],
               s3[:, k:k + 1], None, op0=ALU.mult)

        # ================= layer 4 batch ==================================
        pz4 = pzc.tile([8, BC], f32, name="pz4", tag="ps")
        for k in range(4):
            mm(pz4, vt4[:, 8 * k:8 * k + 8], h3[:, BC * k:BC * k + BC],
               start=(k == 0), stop=(k == 3))
        h4 = wgt.tile([8, BC], f32, name="h4", tag="h4")
        act(h4, pz4, E.Tanh, bias=b4, scale=s4)
        nc.sync.dma_start(t["hT_out"], h4)
        pf4 = pzc.tile([8, BC], f32, name="pf4", tag="ps")
        for k in range(4):
            mm(pf4, vd4[:, 8 * k:8 * k + 8], D3[:, BC * k:BC * k + BC],
               start=(k == 0), stop=(k == 3))
        hq4 = scr.tile([8, BC], f32, name="hq4", tag="hq4")
        mul(hq4, h4, h4)
        s24 = scr.tile([8, BC], f32, name="s24", tag="s24")
        ts(s24, hq4, 1.0, 1.0, op0=ALU.mult, op1=ALU.subtract)   # h4^2 - 1
        gt = scr.tile([8, BC], f32, name="gt", tag="gt")
        stt(gt, pf4, s4, s24, op0=ALU.mult, op1=ALU.mult)
        # fast natural log: ln(x) ~= A*bits(x) + B   (max abs err ~0.03)
        gbits = scr.tile([8, BC], f32, name="gbits", tag="gbits")
        cp(gbits, gt.bitcast(i32))
        sld = wgt.tile([8, BC], f32, name="sld", tag="sld")
        ts(sld, gbits, FASTLN_A, FASTLN_B, op0=ALU.mult, op1=ALU.add)
        nc.sync.dma_start(t["sldT_out"], sld)

    nc.compile()
    return nc


def _host_prep(x, W1, logg1, bias1, W2, logg2, bias2, W3, logg3, bias3,
               W4, logg4, bias4):
    """Pure layout prep (transpose / reshape / gather / masks / casts)."""
    f = np.float32

    def cols(a):          # [512]-ish vector -> [128, 4] column-chunk layout
        return np.ascontiguousarray(np.reshape(a, (4, P)).T).astype(f)

    smalls = np.zeros((P, SMALL_W), f)

    def put(name, arr):
        a, b = _SM[name]
        smalls[:arr.shape[0], a:b] = arr

    def fold(m):          # [512, 8] -> [128, (k x)] with k = row-chunk
        return m.reshape(4, P, 8).transpose(1, 0, 2).reshape(P, 32)

    put("ident", np.eye(P, dtype=f))
    put("w1n", fold(W1))                                   # natural [512,8]
    put("w4t", fold(np.ascontiguousarray(W4.T)))           # [512,8]
    put("w1dg", cols(W1[np.arange(H), np.arange(H) // 64]))
    put("w4dg", cols(W4[np.arange(H) // 64, np.arange(H)]))
    put("lg1", cols(logg1)); put("b1", cols(bias1))
    put("lg2", cols(logg2)); put("b2", cols(bias2))
    put("lg3", cols(logg3)); put("b3", cols(bias3))
    put("lg4c", logg4.reshape(8, 1).astype(f))
    put("b4c", bias4.reshape(8, 1).astype(f))
    # structural masks
    o = np.arange(H)[:, None] // 64
    i1 = np.arange(8)[None, :]
    md1 = (i1 == o).astype(f); mo1 = (i1 < o).astype(f)    # [512, 8] natural
    put("md1n", fold(md1)); put("mo1n", fold(mo1))
    ii = np.arange(H)[:, None] // 64
    o4 = np.arange(8)[None, :]
    md4 = (o4 == ii).astype(f); mo4 = (o4 > ii).astype(f)  # [512, 8] transposed
    put("md4t", fold(md4)); put("mo4t", fold(mo4))

    w2T = np.ascontiguousarray(W2.T).astype(np.float16)
    w3T = np.ascontiguousarray(W3.T).astype(np.float16)
    xT = np.ascontiguousarray(x.T).astype(np.float16)      # [8, 2048]
    return xT, w2T, w3T, smalls


def kernel(**inputs):
    global LAST_RESULTS
    from concourse.bass_utils import run_bass_kernel_spmd

    xT, w2T, w3T, smalls = _host_prep(**{k: np.asarray(v) for k, v in inputs.items()})

    if "nc" not in _CACHE:
        _CACHE["nc"] = _build()
    nc = _CACHE["nc"]

    in_maps = []
    for c in range(NCORE):
        in_maps.append({
            "xT": np.ascontiguousarray(xT[:, BC * c:BC * (c + 1)]),
            "w2T": w2T, "w3T": w3T, "smalls": smalls,
        })
    res = run_bass_kernel_spmd(nc, in_maps, core_ids=list(range(NCORE)),
                               trace=TRACE)
    LAST_RESULTS = res

    B = BC * NCORE
    h = np.empty((B, 8), np.float32)
    sld = np.empty((B, 8), np.float32)
    for c, r in enumerate(res.results):
        h[BC * c:BC * (c + 1)] = r["hT_out"].T
        sld[BC * c:BC * (c + 1)] = r["sldT_out"].T
    return h, sld


# revision 39
# speedup vs baseline: 1.2107x; 1.0296x over previous
"""BNAF forward (B=2048, D=8, H=512, 4 masked layers) on 8 TRN2 NeuronCores.

Strategy
--------
Pure data parallel: batch is split 256/core; the small weights are replicated.

Math: the BNAF log-det recursion collapses in exp space: exp(logdet diag
blocks) == diag blocks of the normalized weight, exp(tanh logdet) == 1-h^2,
so the flow is a chain of positive block-diag matmuls with one log at the
end (2-op DVE fast-log).  The norm scale s=exp(logg)/||v|| is folded
input-side into the next layer's G-flow weights (wd form).  The sech^2
factor is applied as (h^2-1) -- the sign flips cancel across the even
number of layers (with the matching (h4^2-1) fold at L4).

Tile layout notes (trn2 Tile framework tracks deps at TILE granularity, so
false-sharing serializes):
- per-(layer,chunk) PSUM tiles for the h-path matmuls so chunk c+1's MMs
  don't wait on chunk c's tanh (whole-tile WAR).
- weights live in vtO (full rows, DMA-only writers) + vtD (the 4 diag
  128-blocks, strided DMA) per layer, so norm/exp work on vtD never blocks
  reads of the raw off-diag blocks.
- norm^2 columnize is 10 direct lhsT=v^2-window matmuls (no row-sum /
  transpose machinery); one-step Newton rsqrt from a magic seed.
"""

import numpy as np

TRACE = False          # set by test.py for profiling runs
LAST_RESULTS = None    # BassKernelResults stash for test.py

_CACHE = {}

P = 128
BC = 256          # batch per core
H = 512
NCORE = 8
MAGIC = 0x5f3759df

# smalls layout: first the exp-batch block (one ACT op), then the rest
_SM = {}
_off = 0
for _name, _w in [("w1dg", 4), ("w4dg", 4), ("lg1", 4), ("lg2", 4),
                  ("lg3", 4), ("lg4c", 1), ("w1n", 32), ("w4t", 32),  # exp blk
                  ("b4c", 1), ("b1", 4), ("b2", 4), ("b3", 4),
                  ("ident", 128),
                  ("md1n", 32), ("mo1n", 32), ("md4t", 32), ("mo4t", 32)]:
    _SM[_name] = (_off, _off + _w)
    _off += _w
SMALL_W = _off
EXPW = _SM["w4t"][1]           # width of the exp block (85)

FASTLN_A = float(np.log(2.0) / (1 << 23))
FASTLN_B = float((0.0430 - 127.0) * np.log(2.0))


def _build():
    import concourse.bacc as bacc
    import concourse.mybir as mybir
    import concourse.tile as tile
    from concourse.bass import AP
    from contextlib import ExitStack

    f32 = mybir.dt.float32
    u32 = mybir.dt.uint32
    i32 = mybir.dt.int32
    bf16 = mybir.dt.bfloat16
    fp16 = mybir.dt.float16
    E = mybir.ActivationFunctionType
    ALU = mybir.AluOpType

    nc = bacc.Bacc("TRN2", target_bir_lowering=False, debug=False,
                   enable_asserts=False, num_devices=NCORE)

    t = {}
    t["xT"] = nc.dram_tensor("xT", (8, BC), fp16, kind="ExternalInput").ap()
    # host-linearized: row p = [W^T[p,:], W^T[128+p,:], W^T[256+p,:], W^T[384+p,:]]
    t["w2T"] = nc.dram_tensor("w2T", (P, 4 * H), fp16, kind="ExternalInput").ap()
    t["w3T"] = nc.dram_tensor("w3T", (P, 4 * H), fp16, kind="ExternalInput").ap()
    t["smalls"] = nc.dram_tensor("smalls", (P, SMALL_W), f32, kind="ExternalInput").ap()
    t["hT_out"] = nc.dram_tensor("hT_out", (8, BC), f32, kind="ExternalOutput").ap()
    t["sldT_out"] = nc.dram_tensor("sldT_out", (8, BC), f32, kind="ExternalOutput").ap()

    def mm(out, lhsT, rhs, **kw):
        nc.tensor.matmul(out, lhsT, rhs, **kw)

    def winap(base_tile, p0, np_, col0, n, stride, w):
        """[np_ parts at p0] x (n windows of w cols, stride apart, from col0)."""
        base = base_tile[p0:p0 + np_, col0:col0 + w]
        return AP(base.tensor, base.offset,
                  [[base.ap[0][0], np_], [stride, n], [1, w]])

    with tile.TileContext(nc) as tc, ExitStack() as ctx:
        wgt = ctx.enter_context(tc.tile_pool(name="wgt", bufs=1))
        scr = ctx.enter_context(tc.tile_pool(name="scr", bufs=4))
        psN = ctx.enter_context(tc.tile_pool(name="psN", bufs=2, space="PSUM"))
        pzc = ctx.enter_context(tc.tile_pool(name="pzc", bufs=4, space="PSUM"))
        pzg = ctx.enter_context(tc.tile_pool(name="pzg", bufs=1, space="PSUM"))

        act = nc.scalar.activation
        cp = nc.vector.tensor_copy
        ts = nc.vector.tensor_scalar
        stt = nc.vector.scalar_tensor_tensor
        mul = nc.vector.tensor_mul
        tt = nc.vector.tensor_tensor
        gtt = nc.gpsimd.tensor_tensor
        gms = nc.gpsimd.memset

        # ---- dummy ACT at t0 pulls the single exp_and_others table load ---
        dmy = wgt.tile([P, 1], f32, name="dmy", tag="dmy")
        dmyo = wgt.tile([P, 1], f32, name="dmyo", tag="dmyo")
        nc.vector.memset(dmy, 0.0)
        act(dmyo, dmy, E.Exp)
        # PE warmup on zeros: sustained activity through the DMA window so
        # the HAM clock gate opens (1.2 -> 2.4 GHz) before the real stream
        wz = wgt.tile([P, BC + 2], fp16, name="wz", tag="wz")
        pw = psN.tile([2, BC], f32, name="pw", tag="pn")
        nc.vector.memset(wz, 0.0)
        for _ in range(14):
            mm(pw, wz[:, 0:2], wz[:, 2:BC + 2], skip_group_check=True)

        # ---- input DMAs ---------------------------------------------------
        # vtO: full rows (raw W, DMA is the only writer).  vtD: the four
        # diag 128-blocks per layer, chunk c at cols [128c, 128c+128).
        smalls = wgt.tile([P, SMALL_W], f32, name="smalls_t", tag="smalls_t")
        xT = wgt.tile([8, BC], fp16, name="xT_t", tag="xT_t")
        vtO = {l: wgt.tile([P, 4 * H], fp16, name=f"vtO{l}", tag=f"vtO{l}")
               for l in (2, 3)}
        # one diag tile for both layers: block (l, c) at col 512(l-2)+128c
        vtDt = wgt.tile([P, 8 * P], fp16, name="vtD", tag="vtD")
        vtD = {2: vtDt[:, 0:4 * P], 3: vtDt[:, 4 * P:8 * P]}
        nc.sync.dma_start(smalls, t["smalls"])
        for hh in (0, 1):
            cc = slice(2 * H * hh, 2 * H * hh + 2 * H)
            nc.sync.dma_start(vtO[2][:, cc], t["w2T"][:, cc])
        for hh in (0, 1):
            cc = slice(2 * H * hh, 2 * H * hh + 2 * H)
            nc.gpsimd.dma_start(vtO[3][:, cc], t["w3T"][:, cc])
        nc.gpsimd.dma_start(xT, t["xT"])
        # diag 128-blocks: fast on-chip copies out of the full tiles
        for l in (2, 3):
            cp(vtD[l], winap(vtO[l], 0, P, 0, 4, 640, P))

        def sm(name):
            a, b = _SM[name]
            return smalls[:, a:b]

        ident = sm("ident")
        b4 = smalls[0:8, _SM["b4c"][0]:_SM["b4c"][1]]

        # ---- constants ----------------------------------------------------
        ones2f = wgt.tile([P, 2], f32, name="ones2f", tag="ones2f")
        ones2 = wgt.tile([P, 2], fp16, name="ones2", tag="ones2")
        magict = wgt.tile([P, 5], u32, name="magict", tag="magict")
        gms(ones2f, 1.0)
        gms(magict, MAGIC)
        cp(ones2, ones2f)

        # ---- diag prep: zero LL quadrants, exp the 64-blocks (both layers)
        gms(winap(vtDt, 64, 64, 0, 8, P, 64), 0.0)
        dA = winap(vtDt, 0, 64, 0, 8, P, 64)
        dB = winap(vtDt, 64, 64, 64, 8, P, 64)
        act(dA, dA, E.Exp)
        act(dB, dB, E.Exp)

        # ---- one exp over all the small weight pieces ---------------------
        esm = wgt.tile([P, EXPW], f32, name="esm", tag="esm")
        act(esm, smalls[:, 0:EXPW], E.Exp)
        e1n = esm[:, _SM["w1n"][0]:_SM["w1n"][1]]
        e4t = esm[:, _SM["w4t"][0]:_SM["w4t"][1]]
        e1d = esm[:, _SM["w1dg"][0]:_SM["w1dg"][1]]
        eg = {1: esm[:, _SM["lg1"][0]:_SM["lg1"][1]],
              2: esm[:, _SM["lg2"][0]:_SM["lg2"][1]]}
        eg34 = esm[:, _SM["lg3"][0]:_SM["lg4c"][1]]   # [128, 5]: lg3 | lg4c

        # s = eg * rsqrt(n2): magic seed + one Newton step.  The PSUM read
        # is on Vector; the chain arithmetic runs on GpSimd to keep the
        # Vector queue free for the batch-sized ops.
        gts = nc.gpsimd.tensor_scalar
        gstt = nc.gpsimd.scalar_tensor_tensor
        gmul = nc.gpsimd.tensor_mul

        def make_scale(n2_ap, eg_ap, shape, nm, xmul=mul, xts=None, xstt=None):
            xts = xts or ts
            xstt = xstt or stt
            pr = shape[0]
            n2s = scr.tile(list(shape), f32, name=f"n2s_{nm}", tag="sc_n2s")
            cp(n2s, n2_ap)
            shf = scr.tile(list(shape), u32, name=f"shf_{nm}", tag="sc_shf")
            xts(shf, n2s.bitcast(u32), 1, None, op0=ALU.arith_shift_right)
            y0 = scr.tile(list(shape), u32, name=f"y0_{nm}", tag="sc_y0")
            xstt(y0, magict[:pr, :shape[1]], 0, shf, op0=ALU.bypass, op1=ALU.subtract)
            y = y0.bitcast(f32)
            t1 = scr.tile(list(shape), f32, name=f"t1_{nm}", tag="sc_t1")
            t2 = scr.tile(list(shape), f32, name=f"t2_{nm}", tag="sc_t2")
            xmul(t1, y, y)
            xmul(t2, t1, n2s)
            xts(t1, t2, -0.5, 1.5, op0=ALU.mult, op1=ALU.add)
            yn = scr.tile(list(shape), f32, name=f"yn_{nm}", tag="sc_yn")
            xmul(yn, y, t1)
            s = wgt.tile(list(shape), f32, name=f"s_{nm}", tag=f"s_{nm}")
            xmul(s, eg_ap, yn)
            return s

        # ---- v^2 pieces + direct columnized norm matmuls ------------------
        vsqO = {l: scr.tile([P, 4 * H], fp16, name=f"vsqO{l}", tag=f"vsqO{l}")
                for l in (2, 3)}
        vsqDt = scr.tile([P, 8 * P], fp16, name="vsqD", tag="vsqD")
        vsqD = {2: vsqDt[:, 0:4 * P], 3: vsqDt[:, 4 * P:8 * P]}

        def vsq_off(l, engs):
            for k in range(3):
                a, b = H * k + P * (k + 1), H * (k + 1)
                engs[k](vsqO[l][:, a:b], vtO[l][:, a:b], vtO[l][:, a:b],
                        op=ALU.mult)

        def vsq_diag():
            tt(vsqDt, vtDt, vtDt, op=ALU.mult)

        def ncol_direct(l):
            ncol = psN.tile([P, 4], f32, name=f"ncol{l}", tag="pn")
            # off-window contributions first (DMA-gated only), then diag.
            # Only the first-executed MM clears the bank (start=True): all
            # later MMs write-or-accumulate per element via has_written.
            for c in range(1, 4):
                for k in range(c):
                    mm(ncol[:, c:c + 1], vsqO[l][:, H * k + P * c:H * k + P * c + P],
                       ones2[:, 0:1], start=(c == 1 and k == 0), stop=False,
                       skip_group_check=True)
            for c in range(4):
                mm(ncol[:, c:c + 1], vsqD[l][:, P * c:P * c + P], ones2[:, 0:1],
                   start=False, stop=True, skip_group_check=True)
            return ncol

        # wdG: [dA 0; 0 dB] with input-side s_prev fold (per-partition)
        def make_wdG(l, s_prev):
            wdG = wgt.tile([P, 4 * P], fp16, name=f"wdG{l}", tag=f"wdG{l}")
            for c in range(4):
                ts(wdG[:, P * c:P * c + P], vtD[l][:, P * c:P * c + P],
                   s_prev[:, c:c + 1], None, op0=ALU.mult)
            gms(winap(wdG, 0, 64, 64, 4, P, 64), 0.0)   # zero the X quadrant
            return wdG

        # ================= layer 1 prep (natural layout [512,8]) ==========
        v1n = wgt.tile([P, 32], f32, name="v1n", tag="v1n")
        n1 = scr.tile([P, 4], f32, name="n1", tag="n1")
        vT1 = wgt.tile([8, H], fp16, name="vT1", tag="vT1")
        v1a = scr.tile([P, 32], f32, name="v1a", tag="v1a")
        mul(v1a, e1n, sm("md1n"))
        v1b = scr.tile([P, 32], f32, name="v1b", tag="v1b")
        mul(v1b, sm("w1n"), sm("mo1n"))
        tt(v1n, v1a, v1b, op=ALU.add)
        for c in range(4):
            sq1 = scr.tile([P, 8], f32, name=f"sq1_{c}", tag="sq1")
            stt(sq1, v1n[:, 8 * c:8 * c + 8], 0, v1n[:, 8 * c:8 * c + 8],
                op0=ALU.bypass, op1=ALU.mult, accum_out=n1[:, c:c + 1])
        s1 = make_scale(n1, eg[1], (P, 4), "l1")
        ptall = psN.tile([8, H], f32, name="ptall", tag="pn")
        for c in range(4):
            nc.tensor.transpose(ptall[:, P * c:P * c + P],
                                v1n[:, 8 * c:8 * c + 8], ident)
        act(vT1, ptall, E.Copy)     # one PSUM->SBUF copy, on idle Scalar

        # L2 norm chain (s2) -- as early as possible
        vsq_off(2, (tt, gtt, gtt))  # first piece on Vector: earliest PE start
        vsq_diag()
        ncol2 = ncol_direct(2)
        s2 = make_scale(ncol2, eg[2], (P, 4), "l2")

        # ================= layer 1 batch ==================================
        h1 = wgt.tile([P, 4 * BC], fp16, name="h1", tag="h1")
        for c in range(4):
            z = pzc.tile([P, BC], f32, name=f"z1_{c}", tag="ps")
            mm(z, vT1[:, P * c:P * c + P], xT)
            act(h1[:, BC * c:BC * c + BC], z, E.Tanh,
                bias=sm("b1")[:, c:c + 1], scale=s1[:, c:c + 1])
        wdG2 = make_wdG(2, s1)
        # D1 = -F1 = (h1^2 - 1) * e1d  (sign flips cancel across layers)
        hq1 = scr.tile([P, 4 * BC], fp16, name="hq1", tag="hq1")
        tt(hq1, h1, h1, op=ALU.mult)
        D1 = wgt.tile([P, 4 * BC], bf16, name="D1", tag="D1")
        for c in range(4):
            # (hq * e1d) - e1d == e1d * (h^2 - 1)
            ts(D1[:, BC * c:BC * c + BC], hq1[:, BC * c:BC * c + BC],
               e1d[:, c:c + 1], e1d[:, c:c + 1], op0=ALU.mult, op1=ALU.subtract)

        # L3 norm pieces (start early; consumed mid-L2)
        vsq_off(3, (gtt, gtt, gtt))

        # ================= layer 2/3 batch ================================
        def big_batch(l, s, wdG, h_prev, D_prev, mid=None):
            hl = wgt.tile([P, 4 * BC], fp16, name=f"h{l}", tag=f"h{l}")
            zg = pzg.tile([P, 4 * BC], f32, name=f"zg{l}", tag="psg")
            for c in range(4):
                z = pzc.tile([P, BC], f32, name=f"zh{l}_{c}", tag="ps")
                for k in range(c + 1):
                    lhsT = (vtD[l][:, P * c:P * c + P] if k == c else
                            vtO[l][:, H * k + P * c:H * k + P * c + P])
                    mm(z, lhsT, h_prev[:, BC * k:BC * k + BC],
                       start=(k == 0), stop=(k == c))
                act(hl[:, BC * c:BC * c + BC], z, E.Tanh,
                    bias=sm(f"b{l}")[:, c:c + 1], scale=s[:, c:c + 1])
                mm(zg[:, BC * c:BC * c + BC], wdG[:, P * c:P * c + P],
                   D_prev[:, BC * c:BC * c + BC], start=True, stop=True)
                if mid is not None and c == 1:
                    mid()
            hql = scr.tile([P, 4 * BC], fp16, name=f"hq{l}", tag=f"hq{l}")
            Dl = wgt.tile([P, 4 * BC], bf16, name=f"D{l}", tag=f"D{l}")
            if l == 2:
                tt(hql, hl, hl, op=ALU.mult)
            else:
                act(hql, hl, E.Square)
            # D_l = (h^2 - 1) * zg  (zg = wdG @ D_{l-1}, PSUM fp32)
            stt(Dl, hql, 1.0, zg, op0=ALU.subtract, op1=ALU.mult)
            return hl, Dl

        # s3+s4 batched scale chain, emitted mid-L2 via callback
        s34_box = {}

        def mid_l2():
            ncol3 = ncol_direct(3)
            # layer-4 prep
            v4a = scr.tile([P, 32], f32, name="v4a", tag="v4a")
            mul(v4a, e4t, sm("md4t"))
            v4b = scr.tile([P, 32], f32, name="v4b", tag="v4b")
            mul(v4b, sm("w4t"), sm("mo4t"))
            vt4 = wgt.tile([P, 32], fp16, name="vt4", tag="vt4")
            tt(vt4, v4a, v4b, op=ALU.add)
            vsq4 = scr.tile([P, 32], fp16, name="vsq4", tag="vsq4")
            act(vsq4, vt4, E.Square)
            pn4 = psN.tile([8, 2], f32, name="pn4", tag="pn")
            for k in range(4):
                mm(pn4, vsq4[:, 8 * k:8 * k + 8], ones2,
                   start=(k == 0), stop=(k == 3))
            n34 = scr.tile([P, 5], f32, name="n34", tag="n34")
            nc.vector.memset(n34[:, 4:5], 1.0)   # rows 8.. of the s4 col
            cp(n34[:, 0:4], ncol3)
            cp(n34[0:8, 4:5], pn4[:, 0:1])
            s34 = make_scale(n34, eg34, (P, 5), "l34", xmul=gmul)
            s34_box["s3"] = s34[:, 0:4]
            s34_box["s4"] = s34[0:8, 4:5]
            s34_box["v4a"] = v4a
            s34_box["vt4"] = vt4

        h2, D2 = big_batch(2, s2, wdG2, h1, D1, mid=mid_l2)
        s3, s4 = s34_box["s3"], s34_box["s4"]
        v4a, vt4 = s34_box["v4a"], s34_box["vt4"]
        wdG3 = make_wdG(3, s2)

        h3, D3 = big_batch(3, s3, wdG3, h2, D2)

        vd4 = wgt.tile([P, 32], bf16, name="vd4", tag="vd4")
        for k in range(4):
            ts(vd4[:, 8 * k:8 * k + 8], v4a[:, 8 * k:8 * k + 8],
               s3[:, k:k + 1], None, op0=ALU.mult)

        # ================= layer 4 batch ==================================
        pz4 = pzc.tile([8, BC], f32, name="pz4", tag="ps")
        for k in range(4):
            mm(pz4, vt4[:, 8 * k:8 * k + 8], h3[:, BC * k:BC * k + BC],
               start=(k == 0), stop=(k == 3))
        h4 = wgt.tile([8, BC], f32, name="h4", tag="h4")
        act(h4, pz4, E.Tanh, bias=b4, scale=s4)
        nc.sync.dma_start(t["hT_out"], h4)
        pf4 = pzc.tile([8, BC], f32, name="pf4", tag="ps")
        for k in range(4):
            mm(pf4, vd4[:, 8 * k:8 * k + 8], D3[:, BC * k:BC * k + BC],
               start=(k == 0), stop=(k == 3))
        hq4 = scr.tile([8, BC], f32, name="hq4", tag="hq4")
        mul(hq4, h4, h4)
        s24 = scr.tile([8, BC], f32, name="s24", tag="s24")
        ts(s24, hq4, 1.0, 1.0, op0=ALU.mult, op1=ALU.subtract)   # h4^2 - 1
        gt = scr.tile([8, BC], f32, name="gt", tag="gt")
        stt(gt, pf4, s4, s24, op0=ALU.mult, op1=ALU.mult)
        # fast natural log: ln(x) ~= A*bits(x) + B   (max abs err ~0.03)
        gbits = scr.tile([8, BC], f32, name="gbits", tag="gbits")
        cp(gbits, gt.bitcast(i32))
        sld = wgt.tile([8, BC], f32, name="sld", tag="sld")
        ts(sld, gbits, FASTLN_A, FASTLN_B, op0=ALU.mult, op1=ALU.add)
        nc.sync.dma_start(t["sldT_out"], sld)

    nc.compile()
    return nc


def _host_prep(x, W1, logg1, bias1, W2, logg2, bias2, W3, logg3, bias3,
               W4, logg4, bias4):
    """Pure layout prep (transpose / reshape / gather / masks / casts)."""
    f = np.float32

    def cols(a):          # [512]-ish vector -> [128, 4] column-chunk layout
        return np.ascontiguousarray(np.reshape(a, (4, P)).T).astype(f)

    smalls = np.zeros((P, SMALL_W), f)

    def put(name, arr):
        a, b = _SM[name]
        smalls[:arr.shape[0], a:b] = arr

    def fold(m):          # [512, 8] -> [128, (k x)] with k = row-chunk
        return m.reshape(4, P, 8).transpose(1, 0, 2).reshape(P, 32)

    put("ident", np.eye(P, dtype=f))
    put("w1n", fold(W1))                                   # natural [512,8]
    put("w4t", fold(np.ascontiguousarray(W4.T)))           # [512,8]
    put("w1dg", cols(W1[np.arange(H), np.arange(H) // 64]))
    put("w4dg", cols(W4[np.arange(H) // 64, np.arange(H)]))
    put("lg1", cols(logg1)); put("b1", cols(bias1))
    put("lg2", cols(logg2)); put("b2", cols(bias2))
    put("lg3", cols(logg3)); put("b3", cols(bias3))
    put("lg4c", logg4.reshape(8, 1).astype(f))
    put("b4c", bias4.reshape(8, 1).astype(f))
    # structural masks
    o = np.arange(H)[:, None] // 64
    i1 = np.arange(8)[None, :]
    md1 = (i1 == o).astype(f); mo1 = (i1 < o).astype(f)    # [512, 8] natural
    put("md1n", fold(md1)); put("mo1n", fold(mo1))
    ii = np.arange(H)[:, None] // 64
    o4 = np.arange(8)[None, :]
    md4 = (o4 == ii).astype(f); mo4 = (o4 > ii).astype(f)  # [512, 8] transposed
    put("md4t", fold(md4)); put("mo4t", fold(mo4))

    def linz(W):   # [512,512] W -> W.T row-chunks side by side [128, 2048]
        return np.ascontiguousarray(
            W.T.reshape(4, P, H).transpose(1, 0, 2).reshape(P, 4 * H)
        ).astype(np.float16)

    w2T = linz(W2)
    w3T = linz(W3)
    xT = np.ascontiguousarray(x.T).astype(np.float16)      # [8, 2048]
    return xT, w2T, w3T, smalls


def kernel(**inputs):
    global LAST_RESULTS
    from concourse.bass_utils import run_bass_kernel_spmd

    xT, w2T, w3T, smalls = _host_prep(**{k: np.asarray(v) for k, v in inputs.items()})

    if "nc" not in _CACHE:
        _CACHE["nc"] = _build()
    nc = _CACHE["nc"]

    in_maps = []
    for c in range(NCORE):
        in_maps.append({
            "xT": np.ascontiguousarray(xT[:, BC * c:BC * (c + 1)]),
            "w2T": w2T, "w3T": w3T, "smalls": smalls,
        })
    res = run_bass_kernel_spmd(nc, in_maps, core_ids=list(range(NCORE)),
                               trace=TRACE)
    LAST_RESULTS = res

    B = BC * NCORE
    h = np.empty((B, 8), np.float32)
    sld = np.empty((B, 8), np.float32)
    for c, r in enumerate(res.results):
        h[BC * c:BC * (c + 1)] = r["hT_out"].T
        sld[BC * c:BC * (c + 1)] = r["sldT_out"].T
    return h, sld
